# revision 3
# baseline (speedup 1.0000x reference)
"""Multi-head attention (B=2, N=2048, E=1024, H=16, HD=64) on 8 TRN2 NeuronCores.

Sharding: batch (2-way) x head-group (4-way) -> 4 heads per core, full sequence.
Per-core device pipeline:
  x [N,E] fp32 -> cast bf16 -> DRAM bounce -> xbar-transpose -> xT [E,N]
  qT/kT/vT = (W.T @ x.T + b) via PE, W stationary (pair-packed layouts)
  vT -> DRAM bounce -> xbar-transpose -> v [N, 4*HD]
  per head-pair, per q-chunk of 512:
    sT[k,q] = K Q^T (row-packed 2 heads, K=64 each)
    p = exp(sT * scale)  (max-free softmax; scores are O(1) by construction)
    pv[{dA|dB}, q] += v_blk.T @ p (col-packed 2 heads)
    sums[q] += ones.T @ p (M=1 col-packed)
    out_T = pv * broadcast(1/sums)  (K=2 indicator-matmul broadcast on PE)
  out = outT.T @ W_out (pair-major contraction), partial written fp32
Host sums the 4 partials per batch and adds b_out.
"""
import sys

if "/opt/trn_rl_repo" not in sys.path:
    sys.path.insert(0, "/opt/trn_rl_repo")

import numpy as np
import ml_dtypes

B, N, E, H, HD = 2, 2048, 1024, 16, 64
NCORES = 8
HPC = H // 4  # heads per core = 4 (2 pairs)
NT = N // 128  # 16 token tiles
KC = E // 128  # 8 contraction chunks
QC = N // 512  # 4 q chunks
SCALE = HD ** -0.5

_compiled = None


def _build():
    import concourse.bacc as bacc
    import concourse.mybir as mybir
    import concourse.tile as tile

    F32 = mybir.dt.float32
    F32R = mybir.dt.float32r
    BF16 = mybir.dt.bfloat16
    AF = mybir.ActivationFunctionType
    MUL = mybir.AluOpType.mult

    nc = bacc.Bacc("TRN2", target_bir_lowering=False, debug=False,
                   num_devices=NCORES)

    x_d = nc.dram_tensor("x", [N, E], F32, kind="ExternalInput")
    wq_d = nc.dram_tensor("wq", [E, 256], BF16, kind="ExternalInput")
    wk_d = nc.dram_tensor("wk", [E, 256], BF16, kind="ExternalInput")
    wv_d = nc.dram_tensor("wv", [E, 256], BF16, kind="ExternalInput")
    bq_d = nc.dram_tensor("bq", [128, 2], F32, kind="ExternalInput")
    bk_d = nc.dram_tensor("bk", [128, 2], F32, kind="ExternalInput")
    bv_d = nc.dram_tensor("bv", [128, 2], F32, kind="ExternalInput")
    wo_d = nc.dram_tensor("wo", [256, E], BF16, kind="ExternalInput")
    ind_d = nc.dram_tensor("ind", [2, 128], F32, kind="ExternalInput")
    out_d = nc.dram_tensor("out", [N, E], F32, kind="ExternalOutput")

    with tile.TileContext(nc) as tc:
        with (
            tc.tile_pool(name="const", bufs=1) as cp,
            tc.tile_pool(name="dram", bufs=1, space="DRAM") as dp,
            tc.tile_pool(name="xstage", bufs=3) as xp,
            tc.tile_pool(name="ptp", bufs=3) as ptp,
            tc.tile_pool(name="small", bufs=2) as sm,
            tc.tile_pool(name="lin", bufs=2, space="PSUM") as linp,
            tc.tile_pool(name="spsum", bufs=2, space="PSUM") as spp,
            tc.tile_pool(name="pvpsum", bufs=1, space="PSUM") as pvp,
            tc.tile_pool(name="sumpsum", bufs=1, space="PSUM") as sup,
            tc.tile_pool(name="obuf", bufs=3) as obp,
        ):
            # ---------------- constants / weights ----------------
            wq_sb = cp.tile([128, KC * 256], BF16, tag="wq")
            wk_sb = cp.tile([128, KC * 256], BF16, tag="wk")
            wv_sb = cp.tile([128, KC * 256], BF16, tag="wv")
            for w_sb, w_d in ((wq_sb, wq_d), (wk_sb, wk_d), (wv_sb, wv_d)):
                nc.sync.dma_start(
                    w_sb[:].rearrange("p (kc m) -> p kc m", kc=KC),
                    w_d.ap().rearrange("(kc p) m -> p kc m", p=128))
            wo_sb = cp.tile([128, 2 * E], BF16, tag="wo")
            nc.sync.dma_start(
                wo_sb[:].rearrange("p (c m) -> p c m", c=2),
                wo_d.ap().rearrange("(c p) m -> p c m", p=128))
            bq_sb = cp.tile([128, 2], F32, tag="bq")
            bk_sb = cp.tile([128, 2], F32, tag="bk")
            bv_sb = cp.tile([128, 2], F32, tag="bv")
            nc.sync.dma_start(bq_sb[:], bq_d.ap())
            nc.sync.dma_start(bk_sb[:], bk_d.ap())
            nc.sync.dma_start(bv_sb[:], bv_d.ap())
            ones = cp.tile([128, 2], BF16, tag="ones")
            nc.vector.memset(ones[:], 1.0)
            ind32 = cp.tile([2, 128], F32, tag="ind32")
            nc.sync.dma_start(ind32[:], ind_d.ap())
            ind_r = cp.tile([2, 128], F32R, tag="ind_r")
            with nc.allow_low_precision(reason="f32r indicator for broadcast"):
                nc.vector.tensor_copy(ind_r[:], ind32[:])

            # ---------------- x -> xT (cast + DRAM bounce + xbar) ----------------
            xb_dram = dp.tile([N, E], BF16)
            for t in range(NT):
                xt = xp.tile([128, E], F32, tag="xt")
                nc.sync.dma_start(xt[:], x_d.ap()[t * 128:(t + 1) * 128, :])
                xb = xp.tile([128, E], BF16, tag="xb")
                nc.vector.tensor_copy(xb[:], xt[:])
                nc.sync.dma_start(xb_dram[t * 128:(t + 1) * 128, :], xb[:])
            xT = cp.tile([128, KC * N], BF16, tag="xT")
            for ec in range(KC):
                nc.sync.dma_start_transpose(
                    xT[:, ec * N:(ec + 1) * N],
                    xb_dram[:, ec * 128:(ec + 1) * 128])

            # ---------------- qkv projections ----------------
            qT = cp.tile([128, 2 * N], BF16, tag="qT")
            kT = cp.tile([128, 2 * N], BF16, tag="kT")
            vT = cp.tile([128, 2 * N], BF16, tag="vT")
            # emit v first so the v-transpose chain starts early
            for w_sb, b_sb, dstT in ((wv_sb, bv_sb, vT), (wk_sb, bk_sb, kT),
                                     (wq_sb, bq_sb, qT)):
                for p in range(2):
                    for c in range(QC):
                        ps = linp.tile([128, 512], F32, tag="lin")
                        for kc in range(KC):
                            nc.tensor.matmul(
                                ps[:],
                                w_sb[:, kc * 256 + p * 128:kc * 256 + (p + 1) * 128],
                                xT[:, kc * N + c * 512:kc * N + (c + 1) * 512],
                                start=(kc == 0), stop=(kc == KC - 1))
                        nc.vector.tensor_scalar_add(
                            dstT[:, p * N + c * 512:p * N + (c + 1) * 512],
                            ps[:], b_sb[:, p:p + 1])
            vT_dram = dp.tile([256, N], BF16)
            for p in range(2):
                nc.sync.dma_start(vT_dram[p * 128:(p + 1) * 128, :],
                                  vT[:, p * N:(p + 1) * N])
            v_sb = cp.tile([128, NT * 256], BF16, tag="v")
            for t in range(NT):
                nc.sync.dma_start_transpose(
                    v_sb[:, t * 256:(t + 1) * 256],
                    vT_dram[:, t * 128:(t + 1) * 128])

            # ---------------- attention ----------------
            outT0 = cp.tile([128, N], BF16, tag="outT0")
            outT1 = cp.tile([128, N], BF16, tag="outT1")
            outT = [outT0, outT1]
            for hp in range(2):
                for c in range(QC):
                    pv = pvp.tile([128, 512], F32, tag="pv")
                    sums = sup.tile([128, 512], F32, tag="sums")
                    for i in range(NT):
                        s = spp.tile([128, 1024], F32, tag="s")
                        nc.tensor.matmul(
                            s[:, 0:512],
                            kT[0:64, hp * N + i * 128:hp * N + (i + 1) * 128],
                            qT[0:64, hp * N + c * 512:hp * N + (c + 1) * 512],
                            start=True, stop=True)
                        nc.tensor.matmul(
                            s[:, 512:1024],
                            kT[64:128, hp * N + i * 128:hp * N + (i + 1) * 128],
                            qT[64:128, hp * N + c * 512:hp * N + (c + 1) * 512],
                            start=True, stop=True)
                        pT = ptp.tile([128, 1024], BF16, tag="pT")
                        nc.scalar.activation(pT[:], s[:], AF.Exp, scale=SCALE)
                        st, sp_ = (i == 0), (i == NT - 1)
                        nc.tensor.matmul(
                            pv[0:64, :], v_sb[:, i * 256 + hp * 128:i * 256 + hp * 128 + 64],
                            pT[:, 0:512], start=st, stop=sp_, tile_position=(0, 0))
                        nc.tensor.matmul(
                            pv[64:128, :], v_sb[:, i * 256 + hp * 128 + 64:i * 256 + (hp + 1) * 128],
                            pT[:, 512:1024], start=st, stop=sp_, tile_position=(0, 64))
                        nc.tensor.matmul(
                            sums[0:1, :], ones[:, 0:1], pT[:, 0:512],
                            start=st, stop=sp_, tile_position=(0, 0))
                        nc.tensor.matmul(
                            sums[64:65, :], ones[:, 1:2], pT[:, 512:1024],
                            start=st, stop=sp_, tile_position=(0, 64))
                    # normalization: outT = pv * bcast(1/sums)
                    recip = sm.tile([128, 512], F32, tag="recip")
                    nc.vector.reciprocal(recip[0:1, :], sums[0:1, :])
                    nc.vector.reciprocal(recip[64:65, :], sums[64:65, :])
                    r32 = sm.tile([2, 512], F32, tag="r32")
                    nc.vector.tensor_copy(r32[0:1, :], recip[0:1, :])
                    nc.sync.dma_start(r32[1:2, :], recip[64:65, :])
                    rstack = sm.tile([2, 512], F32R, tag="rstack")
                    with nc.allow_low_precision(reason="f32r recip broadcast"):
                        nc.vector.tensor_copy(rstack[:], r32[:])
                    bcp = linp.tile([128, 512], F32, tag="lin")
                    nc.tensor.matmul(bcp[:], ind_r[:], rstack[:],
                                     start=True, stop=True)
                    bc = sm.tile([128, 512], F32, tag="bc")
                    nc.vector.tensor_copy(bc[:], bcp[:])
                    nc.vector.tensor_tensor(
                        outT[hp][:, c * 512:(c + 1) * 512], pv[:], bc[:], op=MUL)

            # ---------------- output projection ----------------
            for t in range(NT):
                for e in range(2):
                    ps = linp.tile([128, 512], F32, tag="lin")
                    for hp in range(2):
                        nc.tensor.matmul(
                            ps[:], outT[hp][:, t * 128:(t + 1) * 128],
                            wo_sb[:, hp * E + e * 512:hp * E + (e + 1) * 512],
                            start=(hp == 0), stop=(hp == 1))
                    ob = obp.tile([128, 512], F32, tag="ob")
                    nc.vector.tensor_copy(ob[:], ps[:])
                    nc.sync.dma_start(
                        out_d.ap()[t * 128:(t + 1) * 128, e * 512:(e + 1) * 512],
                        ob[:])

    nc.compile()
    return nc


def _prep_core_inputs(core, x, W_qkv, b_qkv, W_out):
    bf16 = ml_dtypes.bfloat16
    b, g = divmod(core, 4)  # core = b*4 + g
    heads = [4 * g + j for j in range(HPC)]
    qcols = np.concatenate([np.arange(h * 192, h * 192 + 64) for h in heads])
    kcols = qcols + 64
    vcols = qcols + 128
    wq = np.ascontiguousarray(W_qkv[:, qcols]).astype(bf16)
    wk = np.ascontiguousarray(W_qkv[:, kcols]).astype(bf16)
    wv = np.ascontiguousarray(W_qkv[:, vcols]).astype(bf16)
    bq = np.ascontiguousarray(b_qkv[qcols].reshape(2, 128).T).astype(np.float32)
    bk = np.ascontiguousarray(b_qkv[kcols].reshape(2, 128).T).astype(np.float32)
    bv = np.ascontiguousarray(b_qkv[vcols].reshape(2, 128).T).astype(np.float32)
    orow = np.concatenate([np.arange(h * HD, (h + 1) * HD) for h in heads])
    wo = np.ascontiguousarray(W_out[orow, :]).astype(bf16)
    ind = np.zeros((2, 128), np.float32)
    ind[0, 0:64] = 1.0
    ind[1, 64:128] = 1.0
    return {
        "x": np.ascontiguousarray(x[b]).astype(np.float32),
        "wq": wq, "wk": wk, "wv": wv,
        "bq": bq, "bk": bk, "bv": bv,
        "wo": wo, "ind": ind,
    }


def _get_compiled():
    global _compiled
    if _compiled is None:
        _compiled = _build()
    return _compiled


def run_spmd(x, W_qkv, b_qkv, W_out, b_out, trace=False):
    from concourse.bass_utils import run_bass_kernel_spmd
    nc = _get_compiled()
    in_maps = [_prep_core_inputs(c, x, W_qkv, b_qkv, W_out)
               for c in range(NCORES)]
    res = run_bass_kernel_spmd(nc, in_maps, core_ids=list(range(NCORES)),
                               trace=trace)
    out = np.zeros((B, N, E), np.float32)
    for core in range(NCORES):
        out[core // 4] += res.results[core]["out"]
    out += b_out[None, None, :].astype(np.float32)
    return out, res


def kernel(x, W_qkv, b_qkv, W_out, b_out):
    out, _ = run_spmd(np.asarray(x), np.asarray(W_qkv), np.asarray(b_qkv),
                      np.asarray(W_out), np.asarray(b_out))
    return out


# revision 5
# speedup vs baseline: 1.4166x; 1.4166x over previous
"""Multi-head attention (B=2, N=2048, E=1024, H=16, HD=64) on 8 TRN2 NeuronCores.

Sharding: batch (2-way) x head-group (4-way) -> 4 heads per core (2 pairs),
full sequence per core. Host sums 4 partial out-projections per batch.

Per-core pipeline (all matmuls bf16, fp32 accumulate):
  x -> bf16 -> DRAM bounce -> xbar transpose -> xT [E, N]
  per head-pair p: qT/kT/vT = W.T x.T + b (W stationary, pair-packed);
    v via DRAM bounce + xbar transpose
  attention per (pair, q-chunk c of 512):
    i-loop over 16 k-tiles: sT = K Q^T (row-packed 2 heads),
      p = exp(sT*scale) -> half-strips; pv += v_blk.T p (col-packed)
    sums passes over the strips (ones stationary, LDW-free)
  deferred normalization per pair: batched reciprocal, K=2 indicator-matmul
    broadcast, outT = pv * (1/sums)
  out = outT.T Wout accumulated over pairs -> fp32 partial
"""
import sys

if "/opt/trn_rl_repo" not in sys.path:
    sys.path.insert(0, "/opt/trn_rl_repo")

import numpy as np
import ml_dtypes

B, N, E, H, HD = 2, 2048, 1024, 16, 64
NCORES = 8
HPC = H // 4          # 4 heads per core
NT = N // 128         # 16 k-tiles
KC = E // 128         # 8 contraction chunks
QC = N // 512         # 4 q chunks
SCALE = HD ** -0.5

_compiled = None


def _build():
    import concourse.bacc as bacc
    import concourse.mybir as mybir
    import concourse.tile as tile

    F32 = mybir.dt.float32
    F32R = mybir.dt.float32r
    BF16 = mybir.dt.bfloat16
    AF = mybir.ActivationFunctionType
    MUL = mybir.AluOpType.mult

    nc = bacc.Bacc("TRN2", target_bir_lowering=False, debug=False,
                   num_devices=NCORES)

    x_d = nc.dram_tensor("x", [N, E], F32, kind="ExternalInput")
    wq_d = nc.dram_tensor("wq", [E, 256], BF16, kind="ExternalInput")
    wk_d = nc.dram_tensor("wk", [E, 256], BF16, kind="ExternalInput")
    wv_d = nc.dram_tensor("wv", [E, 256], BF16, kind="ExternalInput")
    bq_d = nc.dram_tensor("bq", [128, 2], F32, kind="ExternalInput")
    bk_d = nc.dram_tensor("bk", [128, 2], F32, kind="ExternalInput")
    bv_d = nc.dram_tensor("bv", [128, 2], F32, kind="ExternalInput")
    wo_d = nc.dram_tensor("wo", [256, E], BF16, kind="ExternalInput")
    ind_d = nc.dram_tensor("ind", [2, 128], F32, kind="ExternalInput")
    out_d = nc.dram_tensor("out", [N, E], F32, kind="ExternalOutput")

    with tile.TileContext(nc) as tc:
        with (
            tc.tile_pool(name="const", bufs=1) as cp,
            tc.tile_pool(name="dram", bufs=1, space="DRAM") as dp,
            tc.tile_pool(name="xstage", bufs=3) as xp,
            tc.tile_pool(name="strip", bufs=3) as stp,
            tc.tile_pool(name="small", bufs=2) as sm,
            tc.tile_pool(name="lin", bufs=2, space="PSUM") as linp,
            tc.tile_pool(name="spsum", bufs=2, space="PSUM") as spp,
            tc.tile_pool(name="pvpsum", bufs=1, space="PSUM") as pvp,
            tc.tile_pool(name="sumpsum", bufs=1, space="PSUM") as sup,
            tc.tile_pool(name="obuf", bufs=3) as obp,
        ):
            # ---------------- weights / constants (issued first) -------------
            wq_sb = cp.tile([128, KC * 256], BF16, tag="wq")
            wk_sb = cp.tile([128, KC * 256], BF16, tag="wk")
            wv_sb = cp.tile([128, KC * 256], BF16, tag="wv")
            for w_sb, w_dr in ((wq_sb, wq_d), (wk_sb, wk_d), (wv_sb, wv_d)):
                nc.sync.dma_start(
                    w_sb[:].rearrange("p (kc m) -> p kc m", kc=KC),
                    w_dr.ap().rearrange("(kc p) m -> p kc m", p=128))
            wo_sb = cp.tile([128, 2 * E], BF16, tag="wo")
            nc.sync.dma_start(
                wo_sb[:].rearrange("p (c m) -> p c m", c=2),
                wo_d.ap().rearrange("(c p) m -> p c m", p=128))
            bq_sb = cp.tile([128, 2], F32, tag="bq")
            bk_sb = cp.tile([128, 2], F32, tag="bk")
            bv_sb = cp.tile([128, 2], F32, tag="bv")
            nc.sync.dma_start(bq_sb[:], bq_d.ap())
            nc.sync.dma_start(bk_sb[:], bk_d.ap())
            nc.sync.dma_start(bv_sb[:], bv_d.ap())
            ones = cp.tile([128, 2], BF16, tag="ones")
            nc.vector.memset(ones[:], 1.0)
            ind32 = cp.tile([2, 128], F32, tag="ind32")
            nc.sync.dma_start(ind32[:], ind_d.ap())
            ind_r = cp.tile([2, 128], F32R, tag="ind_r")
            with nc.allow_low_precision(reason="f32r indicator for broadcast"):
                nc.vector.tensor_copy(ind_r[:], ind32[:])

            # ---------------- x -> xT (cast + DRAM bounce + xbar) ------------
            xb_dram = dp.tile([N, E], BF16)
            for t in range(NT):
                xt = xp.tile([128, E], F32, tag="xt")
                nc.sync.dma_start(xt[:], x_d.ap()[t * 128:(t + 1) * 128, :])
                xb = xp.tile([128, E], BF16, tag="xb")
                nc.vector.tensor_copy(xb[:], xt[:])
                nc.sync.dma_start(xb_dram[t * 128:(t + 1) * 128, :], xb[:])
            xT = cp.tile([128, KC * N], BF16, tag="xT")
            for ec in range(KC):
                for h in range(2):  # row halves -> transpose starts earlier
                    nc.sync.dma_start_transpose(
                        xT[:, ec * N + h * 1024:ec * N + (h + 1) * 1024],
                        xb_dram[h * 1024:(h + 1) * 1024,
                                ec * 128:(ec + 1) * 128])

            qT = cp.tile([128, 2 * N], BF16, tag="qT")
            kT = cp.tile([128, 2 * N], BF16, tag="kT")
            vT = cp.tile([128, 2 * N], BF16, tag="vT")
            v_sb = cp.tile([128, NT * 256], BF16, tag="v")
            vT_dram = dp.tile([256, N], BF16)
            outT0 = cp.tile([128, N], BF16, tag="outT0")
            outT1 = cp.tile([128, N], BF16, tag="outT1")
            outT = [outT0, outT1]
            pv_sb0 = cp.tile([128, N], BF16, tag="pv_sb0")
            pv_sb1 = cp.tile([128, N], BF16, tag="pv_sb1")
            pv_sb = [pv_sb0, pv_sb1]
            sums_sb0 = cp.tile([128, N], F32, tag="sums_sb0")
            sums_sb1 = cp.tile([128, N], F32, tag="sums_sb1")
            sums_sb = [sums_sb0, sums_sb1]
            rst0 = cp.tile([8, 512], F32, tag="rst0")
            rst1 = cp.tile([8, 512], F32, tag="rst1")
            rstage = [rst0, rst1]

            def qkv_pair(p):
                """projections for pair p; vT first so its transpose chain
                starts early."""
                for w_sb, b_sb, dstT in ((wv_sb, bv_sb, vT), (wk_sb, bk_sb, kT),
                                         (wq_sb, bq_sb, qT)):
                    for c in range(QC):
                        ps = linp.tile([128, 512], F32, tag="lin")
                        for kc in range(KC):
                            nc.tensor.matmul(
                                ps[:],
                                w_sb[:, kc * 256 + p * 128:kc * 256 + (p + 1) * 128],
                                xT[:, kc * N + c * 512:kc * N + (c + 1) * 512],
                                start=(kc == 0), stop=(kc == KC - 1))
                        nc.vector.tensor_scalar_add(
                            dstT[:, p * N + c * 512:p * N + (c + 1) * 512],
                            ps[:], b_sb[:, p:p + 1])
                    if dstT is vT:
                        nc.sync.dma_start(vT_dram[p * 128:(p + 1) * 128, :],
                                          vT[:, p * N:(p + 1) * N])
                        for t in range(NT):
                            nc.sync.dma_start_transpose(
                                v_sb[:, t * 256 + p * 128:t * 256 + (p + 1) * 128],
                                vT_dram[p * 128:(p + 1) * 128,
                                        t * 128:(t + 1) * 128])

            def attention_pair(hp):
                for c in range(QC):
                    pv = pvp.tile([128, 512], F32, tag="pv")
                    sums = sup.tile([128, 512], F32, tag="sums")
                    for ih in range(2):  # half strips: i in [8*ih, 8*ih+8)
                        pstrip = stp.tile([128, 8 * 1024], BF16, tag="pstrip")
                        for j in range(8):
                            i = 8 * ih + j
                            s = spp.tile([128, 1024], F32, tag="s")
                            nc.tensor.matmul(
                                s[:, 0:512],
                                kT[0:64, hp * N + i * 128:hp * N + (i + 1) * 128],
                                qT[0:64, hp * N + c * 512:hp * N + (c + 1) * 512],
                                start=True, stop=True)
                            nc.tensor.matmul(
                                s[:, 512:1024],
                                kT[64:128, hp * N + i * 128:hp * N + (i + 1) * 128],
                                qT[64:128, hp * N + c * 512:hp * N + (c + 1) * 512],
                                start=True, stop=True)
                            nc.scalar.activation(
                                pstrip[:, j * 1024:(j + 1) * 1024], s[:],
                                AF.Exp, scale=SCALE)
                            st, sp_ = (i == 0), (i == NT - 1)
                            nc.tensor.matmul(
                                pv[0:64, :],
                                v_sb[:, i * 256 + hp * 128:i * 256 + hp * 128 + 64],
                                pstrip[:, j * 1024:j * 1024 + 512],
                                start=st, stop=sp_, tile_position=(0, 0))
                            nc.tensor.matmul(
                                pv[64:128, :],
                                v_sb[:, i * 256 + hp * 128 + 64:i * 256 + (hp + 1) * 128],
                                pstrip[:, j * 1024 + 512:(j + 1) * 1024],
                                start=st, stop=sp_, tile_position=(0, 64))
                        # dense sums pass over the half strip (ones stationary)
                        for j in range(8):
                            st = (ih == 0 and j == 0)
                            sp_ = (ih == 1 and j == 7)
                            nc.tensor.matmul(
                                sums[0:1, :], ones[:, 0:1],
                                pstrip[:, j * 1024:j * 1024 + 512],
                                start=st, stop=sp_, tile_position=(0, 0))
                            nc.tensor.matmul(
                                sums[64:65, :], ones[:, 1:2],
                                pstrip[:, j * 1024 + 512:(j + 1) * 1024],
                                start=st, stop=sp_, tile_position=(0, 64))
                    # free psum quickly; defer normalization
                    nc.vector.tensor_copy(pv_sb[hp][:, c * 512:(c + 1) * 512],
                                          pv[:])
                    nc.vector.tensor_copy(
                        sums_sb[hp][0:1, c * 512:(c + 1) * 512], sums[0:1, :])
                    nc.vector.tensor_copy(
                        sums_sb[hp][64:65, c * 512:(c + 1) * 512],
                        sums[64:65, :])

            def normalize_pair(hp):
                # gather sums rows into [8, 512] staging: row h*4+c
                rs = rstage[hp]
                for c in range(QC):
                    nc.sync.dma_start(
                        rs[0 + c:1 + c, :],
                        sums_sb[hp][0:1, c * 512:(c + 1) * 512])
                    nc.sync.dma_start(
                        rs[4 + c:5 + c, :],
                        sums_sb[hp][64:65, c * 512:(c + 1) * 512])
                recip = sm.tile([8, 512], F32, tag="recip")
                nc.vector.reciprocal(recip[:], rs[:])
                for c in range(QC):
                    r32 = sm.tile([2, 512], F32, tag="r32")
                    nc.sync.dma_start(r32[0:1, :], recip[c:c + 1, :])
                    nc.sync.dma_start(r32[1:2, :], recip[4 + c:5 + c, :])
                    rstack = sm.tile([2, 512], F32R, tag="rstack")
                    with nc.allow_low_precision(reason="f32r recip broadcast"):
                        nc.vector.tensor_copy(rstack[:], r32[:])
                    bcp = linp.tile([128, 512], F32, tag="lin")
                    nc.tensor.matmul(bcp[:], ind_r[:], rstack[:],
                                     start=True, stop=True)
                    nc.vector.tensor_tensor(
                        outT[hp][:, c * 512:(c + 1) * 512],
                        bcp[:], pv_sb[hp][:, c * 512:(c + 1) * 512], op=MUL)

            qkv_pair(0)
            attention_pair(0)
            qkv_pair(1)
            normalize_pair(0)
            attention_pair(1)
            normalize_pair(1)

            # ---------------- output projection ----------------
            for t in range(NT):
                for e in range(2):
                    ps = linp.tile([128, 512], F32, tag="lin")
                    for hp in range(2):
                        nc.tensor.matmul(
                            ps[:], outT[hp][:, t * 128:(t + 1) * 128],
                            wo_sb[:, hp * E + e * 512:hp * E + (e + 1) * 512],
                            start=(hp == 0), stop=(hp == 1))
                    ob = obp.tile([128, 512], F32, tag="ob")
                    nc.vector.tensor_copy(ob[:], ps[:])
                    nc.sync.dma_start(
                        out_d.ap()[t * 128:(t + 1) * 128, e * 512:(e + 1) * 512],
                        ob[:])

    nc.compile()
    return nc


def _prep_core_inputs(core, x, W_qkv, b_qkv, W_out):
    bf16 = ml_dtypes.bfloat16
    b, g = divmod(core, 4)  # core = b*4 + g
    heads = [4 * g + j for j in range(HPC)]
    qcols = np.concatenate([np.arange(h * 192, h * 192 + 64) for h in heads])
    kcols = qcols + 64
    vcols = qcols + 128
    wq = np.ascontiguousarray(W_qkv[:, qcols]).astype(bf16)
    wk = np.ascontiguousarray(W_qkv[:, kcols]).astype(bf16)
    wv = np.ascontiguousarray(W_qkv[:, vcols]).astype(bf16)
    bq = np.ascontiguousarray(b_qkv[qcols].reshape(2, 128).T).astype(np.float32)
    bk = np.ascontiguousarray(b_qkv[kcols].reshape(2, 128).T).astype(np.float32)
    bv = np.ascontiguousarray(b_qkv[vcols].reshape(2, 128).T).astype(np.float32)
    orow = np.concatenate([np.arange(h * HD, (h + 1) * HD) for h in heads])
    wo = np.ascontiguousarray(W_out[orow, :]).astype(bf16)
    ind = np.zeros((2, 128), np.float32)
    ind[0, 0:64] = 1.0
    ind[1, 64:128] = 1.0
    return {
        "x": np.ascontiguousarray(x[b]).astype(np.float32),
        "wq": wq, "wk": wk, "wv": wv,
        "bq": bq, "bk": bk, "bv": bv,
        "wo": wo, "ind": ind,
    }


def _get_compiled():
    global _compiled
    if _compiled is None:
        _compiled = _build()
    return _compiled


def run_spmd(x, W_qkv, b_qkv, W_out, b_out, trace=False):
    from concourse.bass_utils import run_bass_kernel_spmd
    nc = _get_compiled()
    in_maps = [_prep_core_inputs(c, x, W_qkv, b_qkv, W_out)
               for c in range(NCORES)]
    res = run_bass_kernel_spmd(nc, in_maps, core_ids=list(range(NCORES)),
                               trace=trace)
    out = np.zeros((B, N, E), np.float32)
    for core in range(NCORES):
        out[core // 4] += res.results[core]["out"]
    out += b_out[None, None, :].astype(np.float32)
    return out, res


def kernel(x, W_qkv, b_qkv, W_out, b_out):
    out, _ = run_spmd(np.asarray(x), np.asarray(W_qkv), np.asarray(b_qkv),
                      np.asarray(W_out), np.asarray(b_out))
    return out
